# revision 1
# baseline (speedup 1.0000x reference)
"""Trainium2 Bass kernel for a 2-layer GCN over 2048 independent 25-node
KNN subgraphs (gnn_message_passing).

Strategy:
  - Each 25-node subgraph is independent -> the sparse scatter/gather
    aggregation is a dense per-graph 25x25 matmul. Host packs the
    normalized adjacency (transposed) into block-diagonal 125x125 tiles
    (5 graphs per tile) so the PE array contracts over 125 partitions.
  - Reassociate layer 1: relu(A @ (x @ W0)) == relu((A @ x) @ W0). With
    aggregation first, every matmul stays node-major and no on-chip
    transpose is ever needed (x is fed feature-major from the host).
  - Only the 5 center nodes per tile are needed downstream of the
    layer-2 aggregation, so (A @ h1) is computed for 5 targets only and
    the W1 transform runs once, weight-stationary, over all 260 centers.
  - Data parallel over 8 cores: 256 graphs (52 tiles, last one padded)
    per core; weights replicated.
"""

import os
import sys

import numpy as np

for _p in ("/opt/trn_rl_repo", "/opt/trn_rl_repo/concourse"):
    if _p not in sys.path:
        sys.path.insert(0, _p)

import concourse.bass as bass
import concourse.tile as tile
from concourse import bacc, mybir
from concourse.bass_utils import run_bass_kernel_spmd

NCORES = 8
B = 2048            # graphs
K = 25              # nodes per graph
N = B * K           # 51200
GPC = B // NCORES   # 256 graphs per core
G = 5               # graphs packed per PE tile
P = G * K           # 125 partitions used per tile
NT = (GPC + G - 1) // G   # 52 tiles per core (last tile: 1 real graph)
SLOTS = NT * G      # 260 graph slots per core
NPAD = NT * P       # 6500 padded nodes per core
F0 = 128            # input features
F1 = 256            # hidden features

_f32 = mybir.dt.float32

_compiled = {}


def _build_nc(mm_dt):
    """Build + compile the per-core Bass program. mm_dt is the dtype used
    for SBUF-resident matmul operands (float32 or float32r)."""
    nc = bacc.Bacc("TRN2", target_bir_lowering=False, debug=False,
                   num_devices=NCORES)

    xT_d = nc.dram_tensor("xT", [F0, NPAD], _f32, kind="ExternalInput")
    at_d = nc.dram_tensor("at", [NT, P, P + G], _f32, kind="ExternalInput")
    w0_d = nc.dram_tensor("w0", [F0, F1], _f32, kind="ExternalInput")
    w1_d = nc.dram_tensor("w1", [F1, F1], _f32, kind="ExternalInput")
    wl_d = nc.dram_tensor("wl", [128, 2], _f32, kind="ExternalInput")
    out_d = nc.dram_tensor("out", [1, SLOTS], _f32, kind="ExternalOutput")

    relu = mybir.ActivationFunctionType.Relu

    with tile.TileContext(nc) as tc:
        with (
            tc.tile_pool(name="const", bufs=1) as cpool,
            tc.tile_pool(name="atp", bufs=3) as atp,
            tc.tile_pool(name="qp", bufs=3) as qp,
            tc.tile_pool(name="h1p", bufs=3) as h1p,
            tc.tile_pool(name="outp", bufs=1) as outp,
            tc.tile_pool(name="ps_q", bufs=2, space=bass.MemorySpace.PSUM) as ps_q,
            tc.tile_pool(name="ps_h1", bufs=2, space=bass.MemorySpace.PSUM) as ps_h1,
            tc.tile_pool(name="ps_p2", bufs=2, space=bass.MemorySpace.PSUM) as ps_p2,
            tc.tile_pool(name="ps_f", bufs=1, space=bass.MemorySpace.PSUM) as ps_f,
        ):
            # ---- resident constants ----
            xT = cpool.tile([F0, NPAD], mm_dt)
            nchunk = 4
            cw = NPAD // nchunk
            for c in range(nchunk):
                nc.sync.dma_start(xT[:, c * cw:(c + 1) * cw],
                                  xT_d[:, c * cw:(c + 1) * cw])
            w0 = cpool.tile([F0, F1], mm_dt)
            nc.sync.dma_start(w0[:], w0_d[:])
            w1 = cpool.tile([128, 2 * F1], mm_dt)   # [fic packed] x [fo]
            nc.sync.dma_start(w1[:, 0:F1], w1_d[0:128, :])
            nc.sync.dma_start(w1[:, F1:2 * F1], w1_d[128:256, :])
            wl = cpool.tile([128, 2], mm_dt)
            nc.sync.dma_start(wl[:], wl_d[:])
            p2a = cpool.tile([128, NT, 2, G], mm_dt)   # layer-2 agg (centers), fT-major

            # ---- per-tile loop ----
            for i in range(NT):
                at_t = atp.tile([P, P + G], mm_dt)
                nc.sync.dma_start(at_t[:], at_d[i, :, :])

                # q = x @ W0   (node-major out [125, 256])
                q_ps = ps_q.tile([P, F1], _f32)
                nc.tensor.matmul(q_ps[:], xT[:, i * P:(i + 1) * P], w0[:],
                                 start=True, stop=True)
                q_sb = qp.tile([P, F1], mm_dt)
                nc.vector.tensor_copy(q_sb[:], q_ps[:])

                # h1 = relu(AT.T @ q) = relu(A @ x @ W0)
                h1_ps = ps_h1.tile([P, F1], _f32)
                nc.tensor.matmul(h1_ps[:], at_t[:, 0:P], q_sb[:],
                                 start=True, stop=True)
                h1_sb = h1p.tile([P, F1], mm_dt)
                nc.scalar.activation(h1_sb[:], h1_ps[:], relu)

                # p2cT[f, t] = sum_s h1[s, f] * ATc[s, t]  (5 centers only)
                p2_ps = ps_p2.tile([128, 2, G], _f32)
                for c in range(2):
                    nc.tensor.matmul(p2_ps[:, c, :],
                                     h1_sb[:, c * 128:(c + 1) * 128],
                                     at_t[:, P:P + G],
                                     start=True, stop=True)
                nc.vector.tensor_copy(p2a[:, i, :, :], p2_ps[:, :, :])

            # ---- W1 transform over all centers (weight stationary) ----
            h3_sb = cpool.tile([128, 2, SLOTS], mm_dt)
            for fo in range(2):
                h3_ps = ps_f.tile([128, SLOTS], _f32)
                for fi in range(2):
                    nc.tensor.matmul(h3_ps[:],
                                     w1[:, fi * F1 + fo * 128:fi * F1 + fo * 128 + 128],
                                     p2a[:, :, fi, :],
                                     start=(fi == 0), stop=(fi == 1))
                nc.scalar.activation(h3_sb[:, fo, :], h3_ps[:], relu)

            # ---- out = relu(h3).T @ Wlin ----
            out_ps = ps_f.tile([1, SLOTS], _f32)
            for fo in range(2):
                nc.tensor.matmul(out_ps[:], wl[:, fo:fo + 1], h3_sb[:, fo, :],
                                 start=(fo == 0), stop=(fo == 1))
            out_sb = outp.tile([1, SLOTS], _f32)
            nc.vector.tensor_copy(out_sb[:], out_ps[:])
            nc.sync.dma_start(out_d[:], out_sb[:])

    nc.compile()
    return nc


def _get_nc(use_f32r):
    key = "f32r" if use_f32r else "f32"
    if key not in _compiled:
        dt = mybir.dt.float32r if use_f32r else mybir.dt.float32
        _compiled[key] = _build_nc(dt)
    return _compiled[key]


def _host_prep(x, edge_weight, W0, W1, Wlin, edge_index):
    src = edge_index[0].astype(np.int64)
    tgt = edge_index[1].astype(np.int64)
    b = src // K
    sl = src - b * K
    tl = tgt - (tgt // K) * K

    # dense raw adjacency per graph, indexed [b, t, s]
    idx = (b * K + tl) * K + sl
    Araw = np.bincount(idx, weights=edge_weight.astype(np.float64),
                       minlength=B * K * K).astype(np.float32).reshape(B, K, K)
    deg = Araw.sum(axis=2)                      # weighted in-degree [B, K]
    with np.errstate(divide="ignore"):
        dinv = np.where(deg > 0, 1.0 / np.sqrt(deg), 0.0).astype(np.float32)
    An = Araw * dinv[:, :, None] * dinv[:, None, :]   # [b, t, s]
    ATn = np.ascontiguousarray(An.transpose(0, 2, 1))  # [b, s, t]

    # scatter graphs into per-core padded slots
    ATs = np.zeros((NCORES, SLOTS, K, K), np.float32)
    ATs[:, :GPC] = ATn.reshape(NCORES, GPC, K, K)
    ATs = ATs.reshape(NCORES, NT, G, K, K)

    at = np.zeros((NCORES, NT, P, P + G), np.float32)
    bd = at[..., :P].reshape(NCORES, NT, G, K, G, K)
    cent = at[..., P:].reshape(NCORES, NT, G, K, G)
    for g in range(G):
        bd[:, :, g, :, g, :] = ATs[:, :, g]          # block-diagonal AT
        cent[:, :, g, :, g] = ATs[:, :, g, :, 0]     # center (t_local=0) col

    xp = np.zeros((NCORES, NPAD, F0), np.float32)
    xp[:, :GPC * K] = x.reshape(NCORES, GPC * K, F0)
    xT = np.ascontiguousarray(xp.transpose(0, 2, 1))  # [core, 128, NPAD]

    wl = np.ascontiguousarray(Wlin.reshape(2, 128).T)  # [128, 2]

    in_maps = []
    for c in range(NCORES):
        in_maps.append({
            "xT": xT[c],
            "at": np.ascontiguousarray(at[c]),
            "w0": np.ascontiguousarray(W0.astype(np.float32)),
            "w1": np.ascontiguousarray(W1.astype(np.float32)),
            "wl": wl,
        })
    return in_maps


def _run(inputs, use_f32r=False, trace=False):
    nc = _get_nc(use_f32r)
    in_maps = _host_prep(**inputs)
    res = run_bass_kernel_spmd(nc, in_maps, core_ids=list(range(NCORES)),
                               trace=trace)
    out = np.empty((B, 1), np.float32)
    for c in range(NCORES):
        out[c * GPC:(c + 1) * GPC, 0] = res.results[c]["out"][0, :GPC]
    return out, res


def kernel(**inputs):
    use_f32r = os.environ.get("GCN_F32R", "0") == "1"
    out, _ = _run(inputs, use_f32r=use_f32r, trace=False)
    return out


# revision 5
# speedup vs baseline: 1.2873x; 1.2873x over previous
"""Trainium2 Bass kernel for a 2-layer GCN over 2048 independent 25-node
KNN subgraphs (gnn_message_passing).

Strategy:
  - Each 25-node subgraph is independent -> the sparse scatter/gather
    aggregation is a dense per-graph 25x25 matmul. Host packs the
    normalized adjacency (transposed) into block-diagonal 125x125 tiles
    (5 graphs per tile) so the PE array contracts over 125 partitions.
  - Reassociate layer 1: relu(A @ (x @ W0)) == relu((A @ x) @ W0). With
    aggregation first, every matmul stays node-major and no on-chip
    transpose is ever needed (x is fed feature-major from the host).
  - Only the 5 center nodes per tile are needed downstream of the
    layer-2 aggregation, so (A @ h1) is computed for 5 targets only and
    the W1 transform runs once, weight-stationary, over all 260 centers.
  - Data parallel over 8 cores: 256 graphs (52 tiles, last one padded)
    per core; weights replicated.
"""

import os
import sys

import numpy as np

for _p in ("/opt/trn_rl_repo", "/opt/trn_rl_repo/concourse"):
    if _p not in sys.path:
        sys.path.insert(0, _p)

import concourse.bass as bass
import concourse.tile as tile
from concourse import bacc, mybir
from concourse.bass_utils import run_bass_kernel_spmd

NCORES = 8
B = 2048            # graphs
K = 25              # nodes per graph
N = B * K           # 51200
GPC = B // NCORES   # 256 graphs per core
G = 5               # graphs packed per PE tile
P = G * K           # 125 partitions used per tile
NT = (GPC + G - 1) // G   # 52 tiles per core (last tile: 1 real graph)
SLOTS = NT * G      # 260 graph slots per core
NPAD = NT * P       # 6500 padded nodes per core
F0 = 128            # input features
F1 = 256            # hidden features

_f32 = mybir.dt.float32

_compiled = {}


def _build_nc(mm_dt):
    """Build + compile the per-core Bass program. mm_dt is the dtype used
    for SBUF-resident matmul operands (float32 or float32r)."""
    nc = bacc.Bacc("TRN2", target_bir_lowering=False, debug=False,
                   num_devices=NCORES)

    xT_d = nc.dram_tensor("xT", [F0, NPAD], _f32, kind="ExternalInput")
    # partition-major so the whole tensor DMAs as 125 contiguous 27KB rows
    at_d = nc.dram_tensor("at", [P, NT, P + G], _f32, kind="ExternalInput")
    w0_d = nc.dram_tensor("w0", [F0, F1], _f32, kind="ExternalInput")
    w1_d = nc.dram_tensor("w1", [F1, F1], _f32, kind="ExternalInput")
    wl_d = nc.dram_tensor("wl", [128, 2], _f32, kind="ExternalInput")
    out_d = nc.dram_tensor("out", [1, SLOTS], _f32, kind="ExternalOutput")

    relu = mybir.ActivationFunctionType.Relu

    with tile.TileContext(nc) as tc:
        with (
            tc.tile_pool(name="const", bufs=1) as cpool,
            tc.tile_pool(name="qp", bufs=3) as qp,
            tc.tile_pool(name="h1p", bufs=3) as h1p,
            tc.tile_pool(name="outp", bufs=1) as outp,
            tc.tile_pool(name="ps_q", bufs=2, space=bass.MemorySpace.PSUM) as ps_q,
            tc.tile_pool(name="ps_h1", bufs=2, space=bass.MemorySpace.PSUM) as ps_h1,
            tc.tile_pool(name="ps_p2", bufs=2, space=bass.MemorySpace.PSUM) as ps_p2,
            tc.tile_pool(name="ps_f", bufs=1, space=bass.MemorySpace.PSUM) as ps_f,
        ):
            # ---- resident constants ----
            xT = cpool.tile([F0, NPAD], mm_dt)
            nchunk = 4
            cw = NPAD // nchunk
            for c in range(nchunk):
                nc.sync.dma_start(xT[:, c * cw:(c + 1) * cw],
                                  xT_d[:, c * cw:(c + 1) * cw])
            w0 = cpool.tile([F0, F1], mm_dt)
            nc.sync.dma_start(w0[:], w0_d[:])
            w1 = cpool.tile([128, 2 * F1], mm_dt)   # [fic packed] x [fo]
            nc.sync.dma_start(w1[:, 0:F1], w1_d[0:128, :])
            nc.sync.dma_start(w1[:, F1:2 * F1], w1_d[128:256, :])
            wl = cpool.tile([128, 2], mm_dt)
            nc.sync.dma_start(wl[:], wl_d[:])
            p2a = cpool.tile([128, NT, 2, G], mm_dt)   # layer-2 agg (centers), fT-major

            at_all = cpool.tile([P, NT, P + G], mm_dt)
            nat = 4
            aw = NT // nat
            for c in range(nat):
                nc.sync.dma_start(at_all[:, c * aw:(c + 1) * aw, :],
                                  at_d[:, c * aw:(c + 1) * aw, :])

            # ---- per-tile loop ----
            for i in range(NT):
                at_t = at_all[:, i, :]

                # q = x @ W0   (node-major out [125, 256])
                q_ps = ps_q.tile([P, F1], _f32)
                nc.tensor.matmul(q_ps[:], xT[:, i * P:(i + 1) * P], w0[:],
                                 start=True, stop=True)
                q_sb = qp.tile([P, F1], mm_dt)
                nc.vector.tensor_copy(q_sb[:], q_ps[:])

                # h1 = relu(AT.T @ q) = relu(A @ x @ W0)
                h1_ps = ps_h1.tile([P, F1], _f32)
                nc.tensor.matmul(h1_ps[:], at_t[:, 0:P], q_sb[:],
                                 start=True, stop=True)
                h1_sb = h1p.tile([P, F1], mm_dt)
                nc.scalar.activation(h1_sb[:], h1_ps[:], relu)

                # p2cT[f, t] = sum_s h1[s, f] * ATc[s, t]  (5 centers only)
                p2_ps = ps_p2.tile([128, 2, G], _f32)
                for c in range(2):
                    nc.tensor.matmul(p2_ps[:, c, :],
                                     h1_sb[:, c * 128:(c + 1) * 128],
                                     at_t[:, P:P + G],
                                     start=True, stop=True)
                nc.vector.tensor_copy(p2a[:, i, :, :], p2_ps[:, :, :])

            # ---- W1 transform over all centers (weight stationary) ----
            h3_sb = cpool.tile([128, 2, SLOTS], mm_dt)
            for fo in range(2):
                h3_ps = ps_f.tile([128, SLOTS], _f32)
                for fi in range(2):
                    nc.tensor.matmul(h3_ps[:],
                                     w1[:, fi * F1 + fo * 128:fi * F1 + fo * 128 + 128],
                                     p2a[:, :, fi, :],
                                     start=(fi == 0), stop=(fi == 1))
                nc.scalar.activation(h3_sb[:, fo, :], h3_ps[:], relu)

            # ---- out = relu(h3).T @ Wlin ----
            out_ps = ps_f.tile([1, SLOTS], _f32)
            for fo in range(2):
                nc.tensor.matmul(out_ps[:], wl[:, fo:fo + 1], h3_sb[:, fo, :],
                                 start=(fo == 0), stop=(fo == 1))
            out_sb = outp.tile([1, SLOTS], _f32)
            nc.vector.tensor_copy(out_sb[:], out_ps[:])
            nc.sync.dma_start(out_d[:], out_sb[:])

    nc.compile()
    return nc


def _get_nc(use_f32r):
    key = "f32r" if use_f32r else "f32"
    if key not in _compiled:
        dt = mybir.dt.float32r if use_f32r else mybir.dt.float32
        _compiled[key] = _build_nc(dt)
    return _compiled[key]


def _host_prep(x, edge_weight, W0, W1, Wlin, edge_index):
    src = edge_index[0].astype(np.int64)
    tgt = edge_index[1].astype(np.int64)
    b = src // K
    sl = src - b * K
    tl = tgt - (tgt // K) * K

    # dense raw adjacency per graph, indexed [b, t, s]
    idx = (b * K + tl) * K + sl
    Araw = np.bincount(idx, weights=edge_weight.astype(np.float64),
                       minlength=B * K * K).astype(np.float32).reshape(B, K, K)
    deg = Araw.sum(axis=2)                      # weighted in-degree [B, K]
    with np.errstate(divide="ignore"):
        dinv = np.where(deg > 0, 1.0 / np.sqrt(deg), 0.0).astype(np.float32)
    An = Araw * dinv[:, :, None] * dinv[:, None, :]   # [b, t, s]
    ATn = np.ascontiguousarray(An.transpose(0, 2, 1))  # [b, s, t]

    # scatter graphs into per-core padded slots
    ATs = np.zeros((NCORES, SLOTS, K, K), np.float32)
    ATs[:, :GPC] = ATn.reshape(NCORES, GPC, K, K)
    ATs = ATs.reshape(NCORES, NT, G, K, K)

    at = np.zeros((NCORES, NT, P, P + G), np.float32)
    bd = at[..., :P].reshape(NCORES, NT, G, K, G, K)
    cent = at[..., P:].reshape(NCORES, NT, G, K, G)
    for g in range(G):
        bd[:, :, g, :, g, :] = ATs[:, :, g]          # block-diagonal AT
        cent[:, :, g, :, g] = ATs[:, :, g, :, 0]     # center (t_local=0) col
    # partition-major device layout: [core, 125, NT, 130]
    at = np.ascontiguousarray(at.transpose(0, 2, 1, 3))

    xp = np.zeros((NCORES, NPAD, F0), np.float32)
    xp[:, :GPC * K] = x.reshape(NCORES, GPC * K, F0)
    xT = np.ascontiguousarray(xp.transpose(0, 2, 1))  # [core, 128, NPAD]

    wl = np.ascontiguousarray(Wlin.reshape(2, 128).T)  # [128, 2]

    in_maps = []
    for c in range(NCORES):
        in_maps.append({
            "xT": xT[c],
            "at": np.ascontiguousarray(at[c]),
            "w0": np.ascontiguousarray(W0.astype(np.float32)),
            "w1": np.ascontiguousarray(W1.astype(np.float32)),
            "wl": wl,
        })
    return in_maps


def _run(inputs, use_f32r=False, trace=False):
    nc = _get_nc(use_f32r)
    in_maps = _host_prep(**inputs)
    res = run_bass_kernel_spmd(nc, in_maps, core_ids=list(range(NCORES)),
                               trace=trace)
    out = np.empty((B, 1), np.float32)
    for c in range(NCORES):
        out[c * GPC:(c + 1) * GPC, 0] = res.results[c]["out"][0, :GPC]
    return out, res


def kernel(**inputs):
    use_f32r = os.environ.get("GCN_F32R", "0") == "1"
    out, _ = _run(inputs, use_f32r=use_f32r, trace=False)
    return out


# revision 9
# speedup vs baseline: 1.4912x; 1.1584x over previous
"""Trainium2 Bass kernel for a 2-layer GCN over 2048 independent 25-node
KNN subgraphs (gnn_message_passing).

Strategy:
  - Each 25-node subgraph is independent -> the sparse scatter/gather
    aggregation is a dense per-graph 25x25 matmul. Host packs the
    normalized adjacency (transposed) into block-diagonal 125x125 tiles
    (5 graphs per tile) so the PE array contracts over 125 partitions.
  - Reassociate layer 1: relu(A @ (x @ W0)) == relu((A @ x) @ W0). With
    aggregation first, every matmul stays node-major and no on-chip
    transpose is ever needed (x is fed feature-major from the host).
  - Only the 5 center nodes per tile are needed downstream of the
    layer-2 aggregation, so (A @ h1) is computed for 5 targets only and
    the W1 transform runs once, weight-stationary, over all 260 centers.
  - Data parallel over 8 cores: 256 graphs (52 tiles, last one padded)
    per core; weights replicated.
"""

import os
import sys

import numpy as np

for _p in ("/opt/trn_rl_repo", "/opt/trn_rl_repo/concourse"):
    if _p not in sys.path:
        sys.path.insert(0, _p)

import concourse.bass as bass
import concourse.tile as tile
from concourse import bacc, mybir
from concourse.bass_utils import run_bass_kernel_spmd

NCORES = 8
B = 2048            # graphs
K = 25              # nodes per graph
N = B * K           # 51200
GPC = B // NCORES   # 256 graphs per core
G = 5               # graphs packed per PE tile
P = G * K           # 125 partitions used per tile
NT = (GPC + G - 1) // G   # 52 tiles per core (last tile: 1 real graph)
SLOTS = NT * G      # 260 graph slots per core
NPAD = NT * P       # 6500 padded nodes per core
F0 = 128            # input features
F1 = 256            # hidden features

_f32 = mybir.dt.float32

_compiled = {}


def _build_nc(mm_dt):
    """Build + compile the per-core Bass program. mm_dt is the dtype used
    for SBUF-resident matmul operands (float32 or float32r)."""
    nc = bacc.Bacc("TRN2", target_bir_lowering=False, debug=False,
                   num_devices=NCORES)

    # Inputs declared with the matmul dtype (float32r is bit-identical to
    # f32; np mapping stays float32) so plain DMAs are not dtype casts.
    xT_d = nc.dram_tensor("xT", [F0, NPAD], mm_dt, kind="ExternalInput")
    # partition-major so the whole tensor DMAs as 125 contiguous 27KB rows
    at_d = nc.dram_tensor("at", [P, NT, P + G], mm_dt, kind="ExternalInput")
    w0_d = nc.dram_tensor("w0", [F0, F1], mm_dt, kind="ExternalInput")
    w1_d = nc.dram_tensor("w1", [F1, F1], mm_dt, kind="ExternalInput")
    wl_d = nc.dram_tensor("wl", [128, 2], mm_dt, kind="ExternalInput")
    out_d = nc.dram_tensor("out", [1, SLOTS], _f32, kind="ExternalOutput")

    relu = mybir.ActivationFunctionType.Relu

    with tile.TileContext(nc) as tc:
        with (
            tc.tile_pool(name="const", bufs=1) as cpool,
            tc.tile_pool(name="qp", bufs=3) as qp,
            tc.tile_pool(name="h1p", bufs=3) as h1p,
            tc.tile_pool(name="outp", bufs=1) as outp,
            tc.tile_pool(name="ps_q", bufs=2, space=bass.MemorySpace.PSUM) as ps_q,
            tc.tile_pool(name="ps_h1", bufs=2, space=bass.MemorySpace.PSUM) as ps_h1,
            tc.tile_pool(name="ps_p2", bufs=2, space=bass.MemorySpace.PSUM) as ps_p2,
            tc.tile_pool(name="ps_f", bufs=1, space=bass.MemorySpace.PSUM) as ps_f,
        ):
            # ---- resident constants ----
            xT = cpool.tile([F0, NPAD], mm_dt)
            nchunk = 4
            cw = NPAD // nchunk
            for c in range(nchunk):
                nc.sync.dma_start(xT[:, c * cw:(c + 1) * cw],
                                  xT_d[:, c * cw:(c + 1) * cw])
            w0 = cpool.tile([F0, F1], mm_dt)
            nc.sync.dma_start(w0[:], w0_d[:])
            w1 = cpool.tile([128, 2 * F1], mm_dt)   # [fic packed] x [fo]
            nc.sync.dma_start(w1[:, 0:F1], w1_d[0:128, :])
            nc.sync.dma_start(w1[:, F1:2 * F1], w1_d[128:256, :])
            wl = cpool.tile([128, 2], mm_dt)
            nc.sync.dma_start(wl[:], wl_d[:])
            # layer-2 agg (centers), fT-major, chunk-major so the W1-phase
            # moving operand p2a[:, fi, :] is a contiguous [128, 260]
            p2a = cpool.tile([128, 2, SLOTS], mm_dt)

            at_all = cpool.tile([P, NT, P + G], mm_dt)
            nat = 4
            aw = NT // nat
            for c in range(nat):
                nc.sync.dma_start(at_all[:, c * aw:(c + 1) * aw, :],
                                  at_d[:, c * aw:(c + 1) * aw, :])

            # ---- per-tile loop ----
            for i in range(NT):
                at_t = at_all[:, i, :]

                # q = x @ W0   (node-major out [125, 256])
                q_ps = ps_q.tile([P, F1], _f32)
                nc.tensor.matmul(q_ps[:], xT[:, i * P:(i + 1) * P], w0[:],
                                 start=True, stop=True)
                q_sb = qp.tile([P, F1], mm_dt)
                nc.vector.tensor_copy(q_sb[:], q_ps[:])

                # h1 = relu(AT.T @ q) = relu(A @ x @ W0)
                h1_ps = ps_h1.tile([P, F1], _f32)
                nc.tensor.matmul(h1_ps[:], at_t[:, 0:P], q_sb[:],
                                 start=True, stop=True)
                # keep the tiny 5-wide center aggregation in plain fp32:
                # f32r rejects small/odd moving free-dims at codegen
                h1_sb = h1p.tile([P, F1], _f32)
                nc.scalar.activation(h1_sb[:], h1_ps[:], relu)

                # p2cT[f, t] = sum_s h1[s, f] * ATc[s, t]  (5 centers only)
                atc = at_t[:, P:P + G]
                if mm_dt != _f32:
                    atc = atc.bitcast(_f32)
                p2_ps = ps_p2.tile([128, 2, G], _f32)
                for c in range(2):
                    nc.tensor.matmul(p2_ps[:, c, :],
                                     h1_sb[:, c * 128:(c + 1) * 128],
                                     atc,
                                     start=True, stop=True)
                nc.vector.tensor_copy(p2a[:, :, i * G:(i + 1) * G], p2_ps[:, :, :])

            # ---- W1 transform over all centers (weight stationary) ----
            h3_sb = cpool.tile([128, 2, SLOTS], mm_dt)
            for fo in range(2):
                h3_ps = ps_f.tile([128, SLOTS], _f32)
                for fi in range(2):
                    nc.tensor.matmul(h3_ps[:],
                                     w1[:, fi * F1 + fo * 128:fi * F1 + fo * 128 + 128],
                                     p2a[:, fi, :],
                                     start=(fi == 0), stop=(fi == 1))
                nc.scalar.activation(h3_sb[:, fo, :], h3_ps[:], relu)

            # ---- out = relu(h3).T @ Wlin ----
            out_ps = ps_f.tile([1, SLOTS], _f32)
            for fo in range(2):
                nc.tensor.matmul(out_ps[:], wl[:, fo:fo + 1], h3_sb[:, fo, :],
                                 start=(fo == 0), stop=(fo == 1))
            out_sb = outp.tile([1, SLOTS], _f32)
            nc.vector.tensor_copy(out_sb[:], out_ps[:])
            nc.sync.dma_start(out_d[:], out_sb[:])

    nc.compile()
    return nc


def _get_nc(use_f32r):
    key = "f32r" if use_f32r else "f32"
    if key not in _compiled:
        dt = mybir.dt.float32r if use_f32r else mybir.dt.float32
        _compiled[key] = _build_nc(dt)
    return _compiled[key]


def _host_prep(x, edge_weight, W0, W1, Wlin, edge_index):
    src = edge_index[0].astype(np.int64)
    tgt = edge_index[1].astype(np.int64)
    b = src // K
    sl = src - b * K
    tl = tgt - (tgt // K) * K

    # dense raw adjacency per graph, indexed [b, t, s]
    idx = (b * K + tl) * K + sl
    Araw = np.bincount(idx, weights=edge_weight.astype(np.float64),
                       minlength=B * K * K).astype(np.float32).reshape(B, K, K)
    deg = Araw.sum(axis=2)                      # weighted in-degree [B, K]
    with np.errstate(divide="ignore"):
        dinv = np.where(deg > 0, 1.0 / np.sqrt(deg), 0.0).astype(np.float32)
    An = Araw * dinv[:, :, None] * dinv[:, None, :]   # [b, t, s]
    ATn = np.ascontiguousarray(An.transpose(0, 2, 1))  # [b, s, t]

    # scatter graphs into per-core padded slots
    ATs = np.zeros((NCORES, SLOTS, K, K), np.float32)
    ATs[:, :GPC] = ATn.reshape(NCORES, GPC, K, K)
    ATs = ATs.reshape(NCORES, NT, G, K, K)

    at = np.zeros((NCORES, NT, P, P + G), np.float32)
    bd = at[..., :P].reshape(NCORES, NT, G, K, G, K)
    cent = at[..., P:].reshape(NCORES, NT, G, K, G)
    for g in range(G):
        bd[:, :, g, :, g, :] = ATs[:, :, g]          # block-diagonal AT
        cent[:, :, g, :, g] = ATs[:, :, g, :, 0]     # center (t_local=0) col
    # partition-major device layout: [core, 125, NT, 130]
    at = np.ascontiguousarray(at.transpose(0, 2, 1, 3))

    xp = np.zeros((NCORES, NPAD, F0), np.float32)
    xp[:, :GPC * K] = x.reshape(NCORES, GPC * K, F0)
    xT = np.ascontiguousarray(xp.transpose(0, 2, 1))  # [core, 128, NPAD]

    wl = np.ascontiguousarray(Wlin.reshape(2, 128).T)  # [128, 2]

    in_maps = []
    for c in range(NCORES):
        in_maps.append({
            "xT": xT[c],
            "at": np.ascontiguousarray(at[c]),
            "w0": np.ascontiguousarray(W0.astype(np.float32)),
            "w1": np.ascontiguousarray(W1.astype(np.float32)),
            "wl": wl,
        })
    return in_maps


def _run(inputs, use_f32r=False, trace=False):
    nc = _get_nc(use_f32r)
    in_maps = _host_prep(**inputs)
    res = run_bass_kernel_spmd(nc, in_maps, core_ids=list(range(NCORES)),
                               trace=trace)
    out = np.empty((B, 1), np.float32)
    for c in range(NCORES):
        out[c * GPC:(c + 1) * GPC, 0] = res.results[c]["out"][0, :GPC]
    return out, res


def kernel(**inputs):
    use_f32r = os.environ.get("GCN_F32R", "0") == "1"
    out, _ = _run(inputs, use_f32r=use_f32r, trace=False)
    return out


# revision 10
# speedup vs baseline: 1.8764x; 1.2583x over previous
"""Trainium2 Bass kernel for a 2-layer GCN over 2048 independent 25-node
KNN subgraphs (gnn_message_passing).

Strategy:
  - Each 25-node subgraph is independent -> the sparse scatter/gather
    aggregation is a dense per-graph 25x25 matmul. Host packs the
    normalized adjacency (transposed) into block-diagonal 125x125 tiles
    (5 graphs per tile) so the PE array contracts over 125 partitions.
  - Reassociate layer 1: relu(A @ (x @ W0)) == relu((A @ x) @ W0). With
    aggregation first, every matmul stays node-major and no on-chip
    transpose is ever needed (x is fed feature-major from the host).
  - Only the 5 center nodes per tile are needed downstream of the
    layer-2 aggregation, so (A @ h1) is computed for 5 targets only and
    the W1 transform runs once, weight-stationary, over all 260 centers.
  - Data parallel over 8 cores: 256 graphs (52 tiles, last one padded)
    per core; weights replicated.
"""

import os
import sys

import numpy as np

for _p in ("/opt/trn_rl_repo", "/opt/trn_rl_repo/concourse"):
    if _p not in sys.path:
        sys.path.insert(0, _p)

import concourse.bass as bass
import concourse.tile as tile
from concourse import bacc, mybir
from concourse.bass_utils import run_bass_kernel_spmd

NCORES = 8
B = 2048            # graphs
K = 25              # nodes per graph
N = B * K           # 51200
GPC = B // NCORES   # 256 graphs per core
G = 5               # graphs packed per PE tile
P = G * K           # 125 partitions used per tile
NT = (GPC + G - 1) // G   # 52 tiles per core (last tile: 1 real graph)
SLOTS = NT * G      # 260 graph slots per core
NPAD = NT * P       # 6500 padded nodes per core
CO = 128            # center-column offset inside an at row
CP = 8              # padded center count (f32r needs even moving dims)
AW = CO + CP        # at row width
F0 = 128            # input features
F1 = 256            # hidden features

_f32 = mybir.dt.float32

_compiled = {}


def _build_nc(mm_dt):
    """Build + compile the per-core Bass program. mm_dt is the dtype used
    for SBUF-resident matmul operands (float32 or float32r)."""
    nc = bacc.Bacc("TRN2", target_bir_lowering=False, debug=False,
                   num_devices=NCORES)

    # Inputs declared with the matmul dtype (float32r is bit-identical to
    # f32; np mapping stays float32) so plain DMAs are not dtype casts.
    xT_d = nc.dram_tensor("xT", [F0, NPAD], mm_dt, kind="ExternalInput")
    # partition-major so the whole tensor DMAs as 125 contiguous 27KB rows
    at_d = nc.dram_tensor("at", [P, NT, AW], mm_dt, kind="ExternalInput")
    w0_d = nc.dram_tensor("w0", [F0, F1], mm_dt, kind="ExternalInput")
    w1_d = nc.dram_tensor("w1", [F1, F1], mm_dt, kind="ExternalInput")
    wl_d = nc.dram_tensor("wl", [128, 2], mm_dt, kind="ExternalInput")
    out_d = nc.dram_tensor("out", [1, SLOTS], _f32, kind="ExternalOutput")

    relu = mybir.ActivationFunctionType.Relu

    with tile.TileContext(nc) as tc:
        with (
            tc.tile_pool(name="const", bufs=1) as cpool,
            tc.tile_pool(name="qp", bufs=3) as qp,
            tc.tile_pool(name="h1p", bufs=3) as h1p,
            tc.tile_pool(name="outp", bufs=1) as outp,
            tc.tile_pool(name="ps_q", bufs=2, space=bass.MemorySpace.PSUM) as ps_q,
            tc.tile_pool(name="ps_h1", bufs=2, space=bass.MemorySpace.PSUM) as ps_h1,
            tc.tile_pool(name="ps_p2", bufs=2, space=bass.MemorySpace.PSUM) as ps_p2,
            tc.tile_pool(name="ps_f", bufs=1, space=bass.MemorySpace.PSUM) as ps_f,
        ):
            # ---- resident constants ----
            xT = cpool.tile([F0, NPAD], mm_dt)
            nchunk = 4
            cw = NPAD // nchunk
            for c in range(nchunk):
                nc.sync.dma_start(xT[:, c * cw:(c + 1) * cw],
                                  xT_d[:, c * cw:(c + 1) * cw])
            w0 = cpool.tile([F0, F1], mm_dt)
            nc.sync.dma_start(w0[:], w0_d[:])
            w1 = cpool.tile([128, 2 * F1], mm_dt)   # [fic packed] x [fo]
            nc.sync.dma_start(w1[:, 0:F1], w1_d[0:128, :])
            nc.sync.dma_start(w1[:, F1:2 * F1], w1_d[128:256, :])
            wl = cpool.tile([128, 2], mm_dt)
            nc.sync.dma_start(wl[:], wl_d[:])
            # layer-2 agg (centers), fT-major, chunk-major so the W1-phase
            # moving operand p2a[:, fi, :] is a contiguous [128, 260]
            p2a = cpool.tile([128, 2, SLOTS], mm_dt)

            at_all = cpool.tile([P, NT, AW], mm_dt)
            nat = 4
            aw = NT // nat
            for c in range(nat):
                nc.sync.dma_start(at_all[:, c * aw:(c + 1) * aw, :],
                                  at_d[:, c * aw:(c + 1) * aw, :])

            # ---- per-tile loop ----
            for i in range(NT):
                at_t = at_all[:, i, :]

                # q = x @ W0   (node-major out [125, 256])
                q_ps = ps_q.tile([P, F1], _f32)
                nc.tensor.matmul(q_ps[:], xT[:, i * P:(i + 1) * P], w0[:],
                                 start=True, stop=True)
                q_sb = qp.tile([P, F1], mm_dt)
                nc.vector.tensor_copy(q_sb[:], q_ps[:])

                # h1 = relu(AT.T @ q) = relu(A @ x @ W0)
                h1_ps = ps_h1.tile([P, F1], _f32)
                nc.tensor.matmul(h1_ps[:], at_t[:, 0:P], q_sb[:],
                                 start=True, stop=True)
                h1_sb = h1p.tile([P, F1], mm_dt)
                nc.scalar.activation(h1_sb[:], h1_ps[:], relu)

                # p2cT[f, t] = sum_s h1[s, f] * ATc[s, t] (5 centers, padded
                # to 8 cols: f32r rejects small/odd moving free-dims)
                p2_ps = ps_p2.tile([128, 2, CP], _f32)
                for c in range(2):
                    nc.tensor.matmul(p2_ps[:, c, :],
                                     h1_sb[:, c * 128:(c + 1) * 128],
                                     at_t[:, CO:CO + CP],
                                     start=True, stop=True)
                nc.vector.tensor_copy(p2a[:, :, i * G:(i + 1) * G],
                                      p2_ps[:, :, 0:G])

            # ---- W1 transform over all centers (weight stationary) ----
            h3_sb = cpool.tile([128, 2, SLOTS], mm_dt)
            for fo in range(2):
                h3_ps = ps_f.tile([128, SLOTS], _f32)
                for fi in range(2):
                    nc.tensor.matmul(h3_ps[:],
                                     w1[:, fi * F1 + fo * 128:fi * F1 + fo * 128 + 128],
                                     p2a[:, fi, :],
                                     start=(fi == 0), stop=(fi == 1))
                nc.scalar.activation(h3_sb[:, fo, :], h3_ps[:], relu)

            # ---- out = relu(h3).T @ Wlin ----
            out_ps = ps_f.tile([1, SLOTS], _f32)
            for fo in range(2):
                nc.tensor.matmul(out_ps[:], wl[:, fo:fo + 1], h3_sb[:, fo, :],
                                 start=(fo == 0), stop=(fo == 1))
            out_sb = outp.tile([1, SLOTS], _f32)
            nc.vector.tensor_copy(out_sb[:], out_ps[:])
            nc.sync.dma_start(out_d[:], out_sb[:])

    nc.compile()
    return nc


def _get_nc(use_f32r):
    key = "f32r" if use_f32r else "f32"
    if key not in _compiled:
        dt = mybir.dt.float32r if use_f32r else mybir.dt.float32
        _compiled[key] = _build_nc(dt)
    return _compiled[key]


def _host_prep(x, edge_weight, W0, W1, Wlin, edge_index):
    src = edge_index[0].astype(np.int64)
    tgt = edge_index[1].astype(np.int64)
    b = src // K
    sl = src - b * K
    tl = tgt - (tgt // K) * K

    # dense raw adjacency per graph, indexed [b, t, s]
    idx = (b * K + tl) * K + sl
    Araw = np.bincount(idx, weights=edge_weight.astype(np.float64),
                       minlength=B * K * K).astype(np.float32).reshape(B, K, K)
    deg = Araw.sum(axis=2)                      # weighted in-degree [B, K]
    with np.errstate(divide="ignore"):
        dinv = np.where(deg > 0, 1.0 / np.sqrt(deg), 0.0).astype(np.float32)
    An = Araw * dinv[:, :, None] * dinv[:, None, :]   # [b, t, s]
    ATn = np.ascontiguousarray(An.transpose(0, 2, 1))  # [b, s, t]

    # scatter graphs into per-core padded slots
    ATs = np.zeros((NCORES, SLOTS, K, K), np.float32)
    ATs[:, :GPC] = ATn.reshape(NCORES, GPC, K, K)
    ATs = ATs.reshape(NCORES, NT, G, K, K)

    at = np.zeros((NCORES, NT, P, AW), np.float32)
    bd = at[..., :P].reshape(NCORES, NT, G, K, G, K)
    cent = at[..., CO:CO + G].reshape(NCORES, NT, G, K, G)
    for g in range(G):
        bd[:, :, g, :, g, :] = ATs[:, :, g]          # block-diagonal AT
        cent[:, :, g, :, g] = ATs[:, :, g, :, 0]     # center (t_local=0) col
    # partition-major device layout: [core, 125, NT, 130]
    at = np.ascontiguousarray(at.transpose(0, 2, 1, 3))

    xp = np.zeros((NCORES, NPAD, F0), np.float32)
    xp[:, :GPC * K] = x.reshape(NCORES, GPC * K, F0)
    xT = np.ascontiguousarray(xp.transpose(0, 2, 1))  # [core, 128, NPAD]

    wl = np.ascontiguousarray(Wlin.reshape(2, 128).T)  # [128, 2]

    in_maps = []
    for c in range(NCORES):
        in_maps.append({
            "xT": xT[c],
            "at": np.ascontiguousarray(at[c]),
            "w0": np.ascontiguousarray(W0.astype(np.float32)),
            "w1": np.ascontiguousarray(W1.astype(np.float32)),
            "wl": wl,
        })
    return in_maps


def _run(inputs, use_f32r=False, trace=False):
    nc = _get_nc(use_f32r)
    in_maps = _host_prep(**inputs)
    res = run_bass_kernel_spmd(nc, in_maps, core_ids=list(range(NCORES)),
                               trace=trace)
    out = np.empty((B, 1), np.float32)
    for c in range(NCORES):
        out[c * GPC:(c + 1) * GPC, 0] = res.results[c]["out"][0, :GPC]
    return out, res


def kernel(**inputs):
    use_f32r = os.environ.get("GCN_F32R", "0") == "1"
    out, _ = _run(inputs, use_f32r=use_f32r, trace=False)
    return out
